# revision 19
# baseline (speedup 1.0000x reference)
"""Multi-head attention (B=2, S=4096, E=512, H=8) on 8 trn2 NeuronCores.

Sharding: data-parallel over B (cores 0-3 -> b=0, 4-7 -> b=1) and
sequence-parallel over the query dim (each core owns a 1024-query chunk,
all 8 heads).  Each core computes, for its (b, q-chunk):
    q/k/v projections (bf16), scores^T = k_h q_h^T  (k on partitions,
    two heads row-packed per matmul pair),
    p = exp(scores/sqrt(E)) * mask  (no max-subtraction: |logits| < ~1),
    PV via matmul with a ones-column appended to V (row 64 = softmax denom),
    normalize with a reciprocal broadcast via DRAM bounce,
    output projection (per-head K=64 accumulation) + bias rank-1 matmul.

Host side transposes the per-core input slices and pre-casts matmul
operands to bf16 so every DMA feeds the engines directly.
"""

import math

import ml_dtypes
import numpy as np

B, S, E, H = 2, 4096, 512, 8
HD = E // H  # 64
P = 128
NCORES = 8
QC = (B * S) // NCORES  # 1024 queries per core
NKT = S // P            # 32 k-subtiles of 128
NQT = QC // 512         # 2 q-tiles of 512
NPAIR = H // 2          # 4 head pairs
SCALE = 1.0 / math.sqrt(E)
BF16 = ml_dtypes.bfloat16

_CACHE = {}
LAST_RESULT = None  # BassKernelResults of the most recent run (for test.py)
DEBUG = False  # when True, the built program dumps intermediates


def _build():
    if "nc" in _CACHE:
        return _CACHE["nc"]

    import concourse.bass as bass
    import concourse.tile as tile
    from concourse import bacc, mybir

    f32 = mybir.dt.float32
    bf16 = mybir.dt.bfloat16

    nc = bacc.Bacc(
        "TRN2", target_bir_lowering=False, debug=False, num_devices=NCORES
    )

    maskT = nc.dram_tensor("maskT", [S, QC], bf16, kind="ExternalInput").ap()
    keysT = nc.dram_tensor("keysT", [E, S], bf16, kind="ExternalInput").ap()
    valsT = nc.dram_tensor("valsT", [E, S], bf16, kind="ExternalInput").ap()
    qryT = nc.dram_tensor("qryT", [E, QC], bf16, kind="ExternalInput").ap()
    wqT = nc.dram_tensor("wqT", [E, E], bf16, kind="ExternalInput").ap()
    wkT = nc.dram_tensor("wkT", [E, E], bf16, kind="ExternalInput").ap()
    wvT = nc.dram_tensor("wvT", [E, E], bf16, kind="ExternalInput").ap()
    woT = nc.dram_tensor("woT", [E, E], bf16, kind="ExternalInput").ap()
    bo = nc.dram_tensor("bo", [E], f32, kind="ExternalInput").ap()
    out = nc.dram_tensor("out", [QC, E], f32, kind="ExternalOutput").ap()

    dbg = {}
    if DEBUG:
        dbg["qT"] = nc.dram_tensor("dbg_qT", [P, NPAIR, QC], bf16, kind="ExternalOutput").ap()
        dbg["kT"] = nc.dram_tensor("dbg_kT", [P, NPAIR, S], bf16, kind="ExternalOutput").ap()
        dbg["v"] = nc.dram_tensor("dbg_v", [P, NKT, H, HD + 1], bf16, kind="ExternalOutput").ap()
        dbg["p"] = nc.dram_tensor("dbg_p", [P, 2, 512], f32, kind="ExternalOutput").ap()
        dbg["sc"] = nc.dram_tensor("dbg_sc", [P, 2, 512], f32, kind="ExternalOutput").ap()
        dbg["attn"] = nc.dram_tensor("dbg_attn", [HD, H, QC], bf16, kind="ExternalOutput").ap()
        dbg["den"] = nc.dram_tensor("dbg_den", [1, 512], f32, kind="ExternalOutput").ap()
        dbg["rep"] = nc.dram_tensor("dbg_rep", [HD, 512], f32, kind="ExternalOutput").ap()

    Exp = mybir.ActivationFunctionType.Exp

    with tile.TileContext(nc) as tc:
        with tc.tile_pool(name="persist", bufs=1) as persist:
            # persistent SBUF tensors (per-partition bytes in comments)
            maskb = persist.tile([P, NKT, QC], bf16)         # 64 KB
            v_all = persist.tile([P, NKT, H, HD + 1], bf16)  # 33.3 KB
            kT_all = persist.tile([P, NPAIR, S], bf16)       # 32 KB
            qT_all = persist.tile([P, NPAIR, QC], bf16)      # 8 KB
            attn_all = persist.tile([HD, H, QC], bf16)       # 16 KB
            wo_sb = persist.tile([HD, H, E], bf16)           # 8 KB
            bo_sb = persist.tile([1, E], f32)

            nc.sync.dma_start(
                out=wo_sb, in_=woT.rearrange("(h d) o -> d h o", d=HD)
            )
            nc.sync.dma_start(out=bo_sb, in_=bo[None, :])

            # ---- mask: bf16 0/1, [k-part, kt, q] ----
            # issue on gpsimd's DMA queue to run parallel with other loads
            for kt in range(NKT):
                nc.gpsimd.dma_start(
                    out=maskb[:, kt, :],
                    in_=maskT[kt * P : (kt + 1) * P, :],
                )

            # ---- v projection: all heads at once ----
            with (
                tc.tile_pool(name="wv", bufs=1) as wvp,
                tc.tile_pool(name="vstage", bufs=3) as vstage,
                tc.tile_pool(name="vps", bufs=2, space="PSUM") as vps,
            ):
                wv_sb = wvp.tile([P, 4, E], bf16)
                nc.sync.dma_start(
                    out=wv_sb, in_=wvT.rearrange("(c p) o -> p c o", p=P)
                )
                for kt in range(NKT):
                    vs = vstage.tile([P, 4, P], bf16)
                    nc.scalar.dma_start(
                        out=vs,
                        in_=valsT[:, kt * P : (kt + 1) * P].rearrange(
                            "(c p) s -> p c s", p=P
                        ),
                    )
                    ps = vps.tile([P, E], f32)
                    for ec in range(4):
                        nc.tensor.matmul(
                            ps,
                            lhsT=vs[:, ec, :],
                            rhs=wv_sb[:, ec, :],
                            start=(ec == 0),
                            stop=(ec == 3),
                        )
                    nc.vector.tensor_copy(
                        out=v_all[:, kt, :, 0:HD],
                        in_=ps.rearrange("p (h d) -> p h d", h=H),
                    )
            # ones column for the softmax denominator
            nc.vector.memset(v_all[:, :, :, HD : HD + 1], 1.0)

            # ---- q projection: [pair-d 128, pair, q] ----
            with (
                tc.tile_pool(name="wq", bufs=1) as wqp,
                tc.tile_pool(name="qstage", bufs=2) as qstage,
                tc.tile_pool(name="qps", bufs=2, space="PSUM") as qps,
            ):
                wq_sb = wqp.tile([P, 4, E], bf16)
                nc.sync.dma_start(
                    out=wq_sb, in_=wqT.rearrange("(c p) o -> p c o", p=P)
                )
                for qt in range(NQT):
                    qs = qstage.tile([P, 4, 512], bf16)
                    nc.scalar.dma_start(
                        out=qs,
                        in_=qryT[:, qt * 512 : (qt + 1) * 512].rearrange(
                            "(c p) s -> p c s", p=P
                        ),
                    )
                    for c in range(NPAIR):
                        ps = qps.tile([P, 512], f32)
                        for ec in range(4):
                            nc.tensor.matmul(
                                ps,
                                lhsT=wq_sb[:, ec, c * P : (c + 1) * P],
                                rhs=qs[:, ec, :],
                                start=(ec == 0),
                                stop=(ec == 3),
                            )
                        nc.vector.tensor_copy(
                            out=qT_all[:, c, qt * 512 : (qt + 1) * 512], in_=ps
                        )

            # ---- k projection (interleaved per pair) + attention ----
            with (
                tc.tile_pool(name="wk", bufs=1) as wkp,
                tc.tile_pool(name="kstage", bufs=2) as kstage,
                tc.tile_pool(name="kps", bufs=2, space="PSUM") as kps,
                tc.tile_pool(name="scps", bufs=2, space="PSUM") as scps,
                tc.tile_pool(name="pvps", bufs=2, space="PSUM") as pvps,
                tc.tile_pool(name="pp", bufs=3) as pp,
                tc.tile_pool(name="norm", bufs=2) as norm,
                tc.tile_pool(name="ndram", bufs=2, space="DRAM") as ndram,
            ):
                wk_sb = wkp.tile([P, 4, E], bf16)
                nc.sync.dma_start(
                    out=wk_sb, in_=wkT.rearrange("(c p) o -> p c o", p=P)
                )
                for c in range(NPAIR):
                    # project kT for this pair (overlaps prev pair's attention)
                    for kt8 in range(S // 512):
                        ks = kstage.tile([P, 4, 512], bf16)
                        nc.sync.dma_start(
                            out=ks,
                            in_=keysT[
                                :, kt8 * 512 : (kt8 + 1) * 512
                            ].rearrange("(c2 p) s -> p c2 s", p=P),
                        )
                        ps = kps.tile([P, 512], f32)
                        for ec in range(4):
                            nc.tensor.matmul(
                                ps,
                                lhsT=wk_sb[:, ec, c * P : (c + 1) * P],
                                rhs=ks[:, ec, :],
                                start=(ec == 0),
                                stop=(ec == 3),
                            )
                        nc.vector.tensor_copy(
                            out=kT_all[:, c, kt8 * 512 : (kt8 + 1) * 512],
                            in_=ps,
                        )
                    for qt in range(NQT):
                        qsl = slice(qt * 512, (qt + 1) * 512)
                        pv0 = pvps.tile([HD + 1, 512], f32, tag="pv")
                        pv1 = pvps.tile([HD + 1, 512], f32, tag="pv")
                        for kt in range(NKT):
                            ksl = slice(kt * P, (kt + 1) * P)
                            sc = scps.tile([P, 2, 512], f32)
                            nc.tensor.matmul(
                                sc[:, 0, :],
                                lhsT=kT_all[0:HD, c, ksl],
                                rhs=qT_all[0:HD, c, qsl],
                                start=True,
                                stop=True,
                            )
                            nc.tensor.matmul(
                                sc[:, 1, :],
                                lhsT=kT_all[HD : 2 * HD, c, ksl],
                                rhs=qT_all[HD : 2 * HD, c, qsl],
                                start=True,
                                stop=True,
                            )
                            p_sb = pp.tile([P, 2, 512], bf16)
                            nc.scalar.activation(p_sb, sc, Exp, scale=SCALE)
                            nc.vector.tensor_tensor(
                                out=p_sb[:, 0, :],
                                in0=p_sb[:, 0, :],
                                in1=maskb[:, kt, qsl],
                                op=mybir.AluOpType.mult,
                            )
                            nc.gpsimd.tensor_tensor(
                                out=p_sb[:, 1, :],
                                in0=p_sb[:, 1, :],
                                in1=maskb[:, kt, qsl],
                                op=mybir.AluOpType.mult,
                            )
                            if DEBUG and c == 0 and qt == 0 and kt == 0:
                                dt_ = norm.tile([P, 2, 512], f32, tag="den")
                                nc.vector.tensor_copy(out=dt_, in_=p_sb)
                                nc.sync.dma_start(out=dbg["p"], in_=dt_)
                                dt2 = norm.tile([P, 2, 512], f32, tag="den")
                                nc.vector.tensor_copy(out=dt2, in_=sc)
                                nc.sync.dma_start(out=dbg["sc"], in_=dt2)
                            nc.tensor.matmul(
                                pv0,
                                lhsT=v_all[:, kt, 2 * c, :],
                                rhs=p_sb[:, 0, :],
                                start=(kt == 0),
                                stop=(kt == NKT - 1),
                            )
                            nc.tensor.matmul(
                                pv1,
                                lhsT=v_all[:, kt, 2 * c + 1, :],
                                rhs=p_sb[:, 1, :],
                                start=(kt == 0),
                                stop=(kt == NKT - 1),
                            )
                        for s_, pv in ((0, pv0), (1, pv1)):
                            h = 2 * c + s_
                            den = norm.tile([P, 512], f32, tag="den")
                            nc.vector.tensor_copy(
                                out=den[HD : HD + 1, :],
                                in_=pv[HD : HD + 1, :],
                            )
                            # replicate den across partitions 0..63 via a
                            # DRAM bounce (DRAM sources allow stride-0
                            # partition broadcast APs; SBUF sources don't),
                            # then reciprocal at base partition 0 (the
                            # custom DVE op misbehaves at base 64)
                            dscr = ndram.tile([1, 512], f32, tag="dscr")
                            nc.sync.dma_start(
                                out=dscr, in_=den[HD : HD + 1, :]
                            )
                            den_rep = norm.tile([HD, 512], f32, tag="denr")
                            nc.sync.dma_start(
                                out=den_rep,
                                in_=bass.AP(
                                    tensor=dscr.tensor,
                                    offset=dscr.offset,
                                    ap=[[0, HD], [1, 512]],
                                ),
                            )
                            rep_sb = norm.tile([HD, 512], f32, tag="rep")
                            nc.vector.reciprocal_approx_fast(
                                out=rep_sb, in_=den_rep
                            )
                            if DEBUG and c == 0 and qt == 0 and s_ == 0:
                                nc.sync.dma_start(out=dbg["den"], in_=den[HD : HD + 1, :])
                                nc.sync.dma_start(out=dbg["rep"], in_=rep_sb)
                            nc.vector.tensor_tensor(
                                out=attn_all[:, h, qsl],
                                in0=pv[0:HD, :],
                                in1=rep_sb,
                                op=mybir.AluOpType.mult,
                            )

            if DEBUG:
                for name, tile_src in (("qT", qT_all), ("kT", kT_all), ("v", v_all), ("attn", attn_all)):
                    nc.sync.dma_start(out=dbg[name], in_=tile_src)

            # ---- output projection + bias ----
            with (
                tc.tile_pool(name="ops", bufs=2, space="PSUM") as ops,
                tc.tile_pool(name="osb", bufs=3) as osb,
                tc.tile_pool(name="onesp", bufs=1) as onesp,
            ):
                ones1 = onesp.tile([1, P], f32)
                nc.vector.memset(ones1, 1.0)
                for q8 in range(QC // P):
                    ps = ops.tile([P, E], f32)
                    for h in range(H):
                        nc.tensor.matmul(
                            ps,
                            lhsT=attn_all[:, h, q8 * P : (q8 + 1) * P],
                            rhs=wo_sb[:, h, :],
                            start=(h == 0),
                            stop=False,
                        )
                    # bias via rank-1 matmul: ones^T (1x128) @ bo (1x512)
                    nc.tensor.matmul(
                        ps,
                        lhsT=ones1,
                        rhs=bo_sb,
                        start=False,
                        stop=True,
                    )
                    ob = osb.tile([P, E], f32)
                    nc.vector.tensor_copy(out=ob, in_=ps)
                    nc.gpsimd.dma_start(
                        out=out[q8 * P : (q8 + 1) * P, :], in_=ob
                    )

    nc.compile()
    _CACHE["nc"] = nc
    return nc


def make_in_maps(values, keys, query, mask, Wv, Wk, Wq, Wo, bo):
    values = np.asarray(values, np.float32)
    keys = np.asarray(keys, np.float32)
    query = np.asarray(query, np.float32)
    mask = np.asarray(mask)
    wqT = np.ascontiguousarray(np.asarray(Wq, np.float32).T.astype(BF16))
    wkT = np.ascontiguousarray(np.asarray(Wk, np.float32).T.astype(BF16))
    wvT = np.ascontiguousarray(np.asarray(Wv, np.float32).T.astype(BF16))
    woT = np.ascontiguousarray(np.asarray(Wo, np.float32).T.astype(BF16))
    bo = np.ascontiguousarray(np.asarray(bo, np.float32))

    in_maps = []
    for core in range(NCORES):
        b, qc = core // (NCORES // B), core % (NCORES // B)
        qsl = slice(qc * QC, (qc + 1) * QC)
        in_maps.append(
            {
                "maskT": np.ascontiguousarray(
                    mask[b, 0, qsl, :].T.astype(BF16)
                ),
                "keysT": np.ascontiguousarray(keys[b].T.astype(BF16)),
                "valsT": np.ascontiguousarray(values[b].T.astype(BF16)),
                "qryT": np.ascontiguousarray(query[b, qsl].T.astype(BF16)),
                "wqT": wqT,
                "wkT": wkT,
                "wvT": wvT,
                "woT": woT,
                "bo": bo,
            }
        )
    return in_maps


def kernel(values, keys, query, mask, Wv, Wk, Wq, Wo, bo):
    global LAST_RESULT
    from concourse.bass_utils import run_bass_kernel_spmd

    nc = _build()
    in_maps = make_in_maps(values, keys, query, mask, Wv, Wk, Wq, Wo, bo)
    res = run_bass_kernel_spmd(nc, in_maps, core_ids=list(range(NCORES)))
    LAST_RESULT = res

    out = np.empty((B, S, E), np.float32)
    for core in range(NCORES):
        b, qc = core // (NCORES // B), core % (NCORES // B)
        out[b, qc * QC : (qc + 1) * QC] = res.results[core]["out"]
    return out


# revision 20
# speedup vs baseline: 1.0971x; 1.0971x over previous
"""Multi-head attention (B=2, S=4096, E=512, H=8) on 8 trn2 NeuronCores.

Sharding: data-parallel over B (cores 0-3 -> b=0, 4-7 -> b=1) and
sequence-parallel over the query dim (each core owns a 1024-query chunk,
all 8 heads).  Each core computes, for its (b, q-chunk):
    q/k/v projections (bf16), scores^T = k_h q_h^T  (k on partitions,
    two heads row-packed per matmul pair),
    p = exp(scores/sqrt(E)) * mask  (no max-subtraction: |logits| < ~1),
    PV via matmul with a ones-column appended to V (row 64 = softmax denom),
    normalize with a reciprocal broadcast via DRAM bounce,
    output projection (per-head K=64 accumulation) + bias rank-1 matmul.

Host side transposes the per-core input slices and pre-casts matmul
operands to bf16 so every DMA feeds the engines directly.
"""

import math

import ml_dtypes
import numpy as np

B, S, E, H = 2, 4096, 512, 8
HD = E // H  # 64
P = 128
NCORES = 8
QC = (B * S) // NCORES  # 1024 queries per core
NKT = S // P            # 32 k-subtiles of 128
NQT = QC // 512         # 2 q-tiles of 512
NPAIR = H // 2          # 4 head pairs
SCALE = 1.0 / math.sqrt(E)
BF16 = ml_dtypes.bfloat16

_CACHE = {}
LAST_RESULT = None  # BassKernelResults of the most recent run (for test.py)
DEBUG = False  # when True, the built program dumps intermediates


def _build():
    if "nc" in _CACHE:
        return _CACHE["nc"]

    import concourse.bass as bass
    import concourse.tile as tile
    from concourse import bacc, mybir

    f32 = mybir.dt.float32
    bf16 = mybir.dt.bfloat16

    nc = bacc.Bacc(
        "TRN2", target_bir_lowering=False, debug=False, num_devices=NCORES
    )

    maskT = nc.dram_tensor("maskT", [S, QC], bf16, kind="ExternalInput").ap()
    keysT = nc.dram_tensor("keysT", [E, S], bf16, kind="ExternalInput").ap()
    valsT = nc.dram_tensor("valsT", [E, S], bf16, kind="ExternalInput").ap()
    qryT = nc.dram_tensor("qryT", [E, QC], bf16, kind="ExternalInput").ap()
    wqT = nc.dram_tensor("wqT", [E, E], bf16, kind="ExternalInput").ap()
    wkT = nc.dram_tensor("wkT", [E, E], bf16, kind="ExternalInput").ap()
    wvT = nc.dram_tensor("wvT", [E, E], bf16, kind="ExternalInput").ap()
    woT = nc.dram_tensor("woT", [E, E], bf16, kind="ExternalInput").ap()
    bo = nc.dram_tensor("bo", [E], f32, kind="ExternalInput").ap()
    out = nc.dram_tensor("out", [QC, E], f32, kind="ExternalOutput").ap()

    dbg = {}
    if DEBUG:
        dbg["qT"] = nc.dram_tensor("dbg_qT", [P, NPAIR, QC], bf16, kind="ExternalOutput").ap()
        dbg["kT"] = nc.dram_tensor("dbg_kT", [P, NPAIR, S], bf16, kind="ExternalOutput").ap()
        dbg["v"] = nc.dram_tensor("dbg_v", [P, NKT, H, HD + 1], bf16, kind="ExternalOutput").ap()
        dbg["p"] = nc.dram_tensor("dbg_p", [P, 2, 512], f32, kind="ExternalOutput").ap()
        dbg["sc"] = nc.dram_tensor("dbg_sc", [P, 2, 512], f32, kind="ExternalOutput").ap()
        dbg["attn"] = nc.dram_tensor("dbg_attn", [HD, H, QC], bf16, kind="ExternalOutput").ap()
        dbg["den"] = nc.dram_tensor("dbg_den", [1, 512], f32, kind="ExternalOutput").ap()
        dbg["rep"] = nc.dram_tensor("dbg_rep", [HD, 512], f32, kind="ExternalOutput").ap()

    Exp = mybir.ActivationFunctionType.Exp

    with tile.TileContext(nc) as tc:
        with tc.tile_pool(name="persist", bufs=1) as persist:
            # persistent SBUF tensors (per-partition bytes in comments)
            maskb = persist.tile([P, NKT, QC], bf16)         # 64 KB
            v_all = persist.tile([P, NKT, H, HD + 1], bf16)  # 33.3 KB
            kT_all = persist.tile([P, NPAIR, S], bf16)       # 32 KB
            qT_all = persist.tile([P, NPAIR, QC], bf16)      # 8 KB
            attn_all = persist.tile([HD, H, QC], bf16)       # 16 KB
            wo_sb = persist.tile([HD, H, E], bf16)           # 8 KB
            bo_sb = persist.tile([1, E], f32)

            nc.sync.dma_start(
                out=wo_sb, in_=woT.rearrange("(h d) o -> d h o", d=HD)
            )
            nc.sync.dma_start(out=bo_sb, in_=bo[None, :])

            # ---- mask: bf16 0/1, [k-part, kt, q] ----
            # issue on gpsimd's DMA queue to run parallel with other loads
            for kt in range(NKT):
                nc.gpsimd.dma_start(
                    out=maskb[:, kt, :],
                    in_=maskT[kt * P : (kt + 1) * P, :],
                )

            # ---- v projection: all heads at once ----
            with (
                tc.tile_pool(name="wv", bufs=1) as wvp,
                tc.tile_pool(name="vstage", bufs=3) as vstage,
                tc.tile_pool(name="vps", bufs=2, space="PSUM") as vps,
            ):
                wv_sb = wvp.tile([P, 4, E], bf16)
                nc.sync.dma_start(
                    out=wv_sb, in_=wvT.rearrange("(c p) o -> p c o", p=P)
                )
                for kt in range(NKT):
                    vs = vstage.tile([P, 4, P], bf16)
                    nc.scalar.dma_start(
                        out=vs,
                        in_=valsT[:, kt * P : (kt + 1) * P].rearrange(
                            "(c p) s -> p c s", p=P
                        ),
                    )
                    ps = vps.tile([P, E], f32)
                    for ec in range(4):
                        nc.tensor.matmul(
                            ps,
                            lhsT=vs[:, ec, :],
                            rhs=wv_sb[:, ec, :],
                            start=(ec == 0),
                            stop=(ec == 3),
                        )
                    nc.vector.tensor_copy(
                        out=v_all[:, kt, :, 0:HD],
                        in_=ps.rearrange("p (h d) -> p h d", h=H),
                    )
            # ones column for the softmax denominator
            nc.vector.memset(v_all[:, :, :, HD : HD + 1], 1.0)

            # ---- q projection: [pair-d 128, pair, q] ----
            with (
                tc.tile_pool(name="wq", bufs=1) as wqp,
                tc.tile_pool(name="qstage", bufs=2) as qstage,
                tc.tile_pool(name="qps", bufs=2, space="PSUM") as qps,
            ):
                wq_sb = wqp.tile([P, 4, E], bf16)
                nc.sync.dma_start(
                    out=wq_sb, in_=wqT.rearrange("(c p) o -> p c o", p=P)
                )
                for qt in range(NQT):
                    qs = qstage.tile([P, 4, 512], bf16)
                    nc.scalar.dma_start(
                        out=qs,
                        in_=qryT[:, qt * 512 : (qt + 1) * 512].rearrange(
                            "(c p) s -> p c s", p=P
                        ),
                    )
                    for c in range(NPAIR):
                        ps = qps.tile([P, 512], f32)
                        for ec in range(4):
                            nc.tensor.matmul(
                                ps,
                                lhsT=wq_sb[:, ec, c * P : (c + 1) * P],
                                rhs=qs[:, ec, :],
                                start=(ec == 0),
                                stop=(ec == 3),
                            )
                        nc.vector.tensor_copy(
                            out=qT_all[:, c, qt * 512 : (qt + 1) * 512], in_=ps
                        )

            # ---- k projection (interleaved per pair) + attention ----
            with (
                tc.tile_pool(name="wk", bufs=1) as wkp,
                tc.tile_pool(name="kstage", bufs=2) as kstage,
                tc.tile_pool(name="kps", bufs=2, space="PSUM") as kps,
                tc.tile_pool(name="scps", bufs=2, space="PSUM") as scps,
                tc.tile_pool(name="pvps", bufs=2, space="PSUM") as pvps,
                tc.tile_pool(name="pp", bufs=3) as pp,
                tc.tile_pool(name="norm", bufs=2) as norm,
                tc.tile_pool(name="ndram", bufs=2, space="DRAM") as ndram,
            ):
                wk_sb = wkp.tile([P, 4, E], bf16)
                nc.sync.dma_start(
                    out=wk_sb, in_=wkT.rearrange("(c p) o -> p c o", p=P)
                )
                for c in range(NPAIR):
                    # project kT for this pair (overlaps prev pair's attention)
                    for kt8 in range(S // 512):
                        ks = kstage.tile([P, 4, 512], bf16)
                        nc.sync.dma_start(
                            out=ks,
                            in_=keysT[
                                :, kt8 * 512 : (kt8 + 1) * 512
                            ].rearrange("(c2 p) s -> p c2 s", p=P),
                        )
                        ps = kps.tile([P, 512], f32)
                        for ec in range(4):
                            nc.tensor.matmul(
                                ps,
                                lhsT=wk_sb[:, ec, c * P : (c + 1) * P],
                                rhs=ks[:, ec, :],
                                start=(ec == 0),
                                stop=(ec == 3),
                            )
                        nc.vector.tensor_copy(
                            out=kT_all[:, c, kt8 * 512 : (kt8 + 1) * 512],
                            in_=ps,
                        )
                    for qt in range(NQT):
                        qsl = slice(qt * 512, (qt + 1) * 512)
                        pv0 = pvps.tile([HD + 1, 512], f32, tag="pv")
                        pv1 = pvps.tile([HD + 1, 512], f32, tag="pv")
                        for kt in range(NKT):
                            ksl = slice(kt * P, (kt + 1) * P)
                            sc = scps.tile([P, 2, 512], f32)
                            nc.tensor.matmul(
                                sc[:, 0, :],
                                lhsT=kT_all[0:HD, c, ksl],
                                rhs=qT_all[0:HD, c, qsl],
                                start=True,
                                stop=True,
                            )
                            nc.tensor.matmul(
                                sc[:, 1, :],
                                lhsT=kT_all[HD : 2 * HD, c, ksl],
                                rhs=qT_all[HD : 2 * HD, c, qsl],
                                start=True,
                                stop=True,
                            )
                            p_sb = pp.tile([P, 2, 512], bf16)
                            nc.scalar.activation(p_sb, sc, Exp, scale=SCALE)
                            for s_ in range(2):
                                nc.vector.tensor_tensor(
                                    out=p_sb[:, s_, :],
                                    in0=p_sb[:, s_, :],
                                    in1=maskb[:, kt, qsl],
                                    op=mybir.AluOpType.mult,
                                )
                            if DEBUG and c == 0 and qt == 0 and kt == 0:
                                dt_ = norm.tile([P, 2, 512], f32, tag="den")
                                nc.vector.tensor_copy(out=dt_, in_=p_sb)
                                nc.sync.dma_start(out=dbg["p"], in_=dt_)
                                dt2 = norm.tile([P, 2, 512], f32, tag="den")
                                nc.vector.tensor_copy(out=dt2, in_=sc)
                                nc.sync.dma_start(out=dbg["sc"], in_=dt2)
                            nc.tensor.matmul(
                                pv0,
                                lhsT=v_all[:, kt, 2 * c, :],
                                rhs=p_sb[:, 0, :],
                                start=(kt == 0),
                                stop=(kt == NKT - 1),
                            )
                            nc.tensor.matmul(
                                pv1,
                                lhsT=v_all[:, kt, 2 * c + 1, :],
                                rhs=p_sb[:, 1, :],
                                start=(kt == 0),
                                stop=(kt == NKT - 1),
                            )
                        for s_, pv in ((0, pv0), (1, pv1)):
                            h = 2 * c + s_
                            den = norm.tile([P, 512], f32, tag="den")
                            nc.vector.tensor_copy(
                                out=den[HD : HD + 1, :],
                                in_=pv[HD : HD + 1, :],
                            )
                            # replicate den across partitions 0..63 via a
                            # DRAM bounce (DRAM sources allow stride-0
                            # partition broadcast APs; SBUF sources don't),
                            # then reciprocal at base partition 0 (the
                            # custom DVE op misbehaves at base 64)
                            dscr = ndram.tile([1, 512], f32, tag="dscr")
                            nc.sync.dma_start(
                                out=dscr, in_=den[HD : HD + 1, :]
                            )
                            den_rep = norm.tile([HD, 512], f32, tag="denr")
                            nc.sync.dma_start(
                                out=den_rep,
                                in_=bass.AP(
                                    tensor=dscr.tensor,
                                    offset=dscr.offset,
                                    ap=[[0, HD], [1, 512]],
                                ),
                            )
                            rep_sb = norm.tile([HD, 512], f32, tag="rep")
                            nc.vector.reciprocal_approx_fast(
                                out=rep_sb, in_=den_rep
                            )
                            if DEBUG and c == 0 and qt == 0 and s_ == 0:
                                nc.sync.dma_start(out=dbg["den"], in_=den[HD : HD + 1, :])
                                nc.sync.dma_start(out=dbg["rep"], in_=rep_sb)
                            nc.vector.tensor_tensor(
                                out=attn_all[:, h, qsl],
                                in0=pv[0:HD, :],
                                in1=rep_sb,
                                op=mybir.AluOpType.mult,
                            )

            if DEBUG:
                for name, tile_src in (("qT", qT_all), ("kT", kT_all), ("v", v_all), ("attn", attn_all)):
                    nc.sync.dma_start(out=dbg[name], in_=tile_src)

            # ---- output projection + bias ----
            with (
                tc.tile_pool(name="ops", bufs=2, space="PSUM") as ops,
                tc.tile_pool(name="osb", bufs=3) as osb,
                tc.tile_pool(name="onesp", bufs=1) as onesp,
            ):
                ones1 = onesp.tile([1, P], f32)
                nc.vector.memset(ones1, 1.0)
                for q8 in range(QC // P):
                    ps = ops.tile([P, E], f32)
                    for h in range(H):
                        nc.tensor.matmul(
                            ps,
                            lhsT=attn_all[:, h, q8 * P : (q8 + 1) * P],
                            rhs=wo_sb[:, h, :],
                            start=(h == 0),
                            stop=False,
                        )
                    # bias via rank-1 matmul: ones^T (1x128) @ bo (1x512)
                    nc.tensor.matmul(
                        ps,
                        lhsT=ones1,
                        rhs=bo_sb,
                        start=False,
                        stop=True,
                    )
                    ob = osb.tile([P, E], f32)
                    nc.vector.tensor_copy(out=ob, in_=ps)
                    nc.gpsimd.dma_start(
                        out=out[q8 * P : (q8 + 1) * P, :], in_=ob
                    )

    nc.compile()
    _CACHE["nc"] = nc
    return nc


def make_in_maps(values, keys, query, mask, Wv, Wk, Wq, Wo, bo):
    values = np.asarray(values, np.float32)
    keys = np.asarray(keys, np.float32)
    query = np.asarray(query, np.float32)
    mask = np.asarray(mask)
    wqT = np.ascontiguousarray(np.asarray(Wq, np.float32).T.astype(BF16))
    wkT = np.ascontiguousarray(np.asarray(Wk, np.float32).T.astype(BF16))
    wvT = np.ascontiguousarray(np.asarray(Wv, np.float32).T.astype(BF16))
    woT = np.ascontiguousarray(np.asarray(Wo, np.float32).T.astype(BF16))
    bo = np.ascontiguousarray(np.asarray(bo, np.float32))

    in_maps = []
    for core in range(NCORES):
        b, qc = core // (NCORES // B), core % (NCORES // B)
        qsl = slice(qc * QC, (qc + 1) * QC)
        in_maps.append(
            {
                "maskT": np.ascontiguousarray(
                    mask[b, 0, qsl, :].T.astype(BF16)
                ),
                "keysT": np.ascontiguousarray(keys[b].T.astype(BF16)),
                "valsT": np.ascontiguousarray(values[b].T.astype(BF16)),
                "qryT": np.ascontiguousarray(query[b, qsl].T.astype(BF16)),
                "wqT": wqT,
                "wkT": wkT,
                "wvT": wvT,
                "woT": woT,
                "bo": bo,
            }
        )
    return in_maps


def kernel(values, keys, query, mask, Wv, Wk, Wq, Wo, bo):
    global LAST_RESULT
    from concourse.bass_utils import run_bass_kernel_spmd

    nc = _build()
    in_maps = make_in_maps(values, keys, query, mask, Wv, Wk, Wq, Wo, bo)
    res = run_bass_kernel_spmd(nc, in_maps, core_ids=list(range(NCORES)))
    LAST_RESULT = res

    out = np.empty((B, S, E), np.float32)
    for core in range(NCORES):
        b, qc = core // (NCORES // B), core % (NCORES // B)
        out[b, qc * QC : (qc + 1) * QC] = res.results[core]["out"]
    return out


# revision 21
# speedup vs baseline: 1.1842x; 1.0794x over previous
"""Multi-head attention (B=2, S=4096, E=512, H=8) on 8 trn2 NeuronCores.

Sharding: data-parallel over B (cores 0-3 -> b=0, 4-7 -> b=1) and
sequence-parallel over the query dim (each core owns a 1024-query chunk,
all 8 heads).  Each core computes, for its (b, q-chunk):
    q/k/v projections (bf16), scores^T = k_h q_h^T  (k on partitions,
    two heads row-packed per matmul pair),
    p = exp(scores/sqrt(E)) * mask  (no max-subtraction: |logits| < ~1),
    PV via matmul with a ones-column appended to V (row 64 = softmax denom),
    normalize with a reciprocal broadcast via DRAM bounce,
    output projection (per-head K=64 accumulation) + bias rank-1 matmul.

Host side transposes the per-core input slices and pre-casts matmul
operands to bf16 so every DMA feeds the engines directly.
"""

import math

import ml_dtypes
import numpy as np

B, S, E, H = 2, 4096, 512, 8
HD = E // H  # 64
P = 128
NCORES = 8
QC = (B * S) // NCORES  # 1024 queries per core
NKT = S // P            # 32 k-subtiles of 128
NQT = QC // 512         # 2 q-tiles of 512
NPAIR = H // 2          # 4 head pairs
SCALE = 1.0 / math.sqrt(E)
BF16 = ml_dtypes.bfloat16

_CACHE = {}
LAST_RESULT = None  # BassKernelResults of the most recent run (for test.py)
DEBUG = False  # when True, the built program dumps intermediates


def _build():
    if "nc" in _CACHE:
        return _CACHE["nc"]

    import concourse.bass as bass
    import concourse.tile as tile
    from concourse import bacc, mybir

    f32 = mybir.dt.float32
    bf16 = mybir.dt.bfloat16

    nc = bacc.Bacc(
        "TRN2", target_bir_lowering=False, debug=False, num_devices=NCORES
    )

    maskT = nc.dram_tensor("maskT", [S, QC], bf16, kind="ExternalInput").ap()
    keysT = nc.dram_tensor("keysT", [E, S], bf16, kind="ExternalInput").ap()
    valsT = nc.dram_tensor("valsT", [E, S], bf16, kind="ExternalInput").ap()
    qryT = nc.dram_tensor("qryT", [E, QC], bf16, kind="ExternalInput").ap()
    wqT = nc.dram_tensor("wqT", [E, E], bf16, kind="ExternalInput").ap()
    wkT = nc.dram_tensor("wkT", [E, E], bf16, kind="ExternalInput").ap()
    wvT = nc.dram_tensor("wvT", [E, E], bf16, kind="ExternalInput").ap()
    woT = nc.dram_tensor("woT", [E, E], bf16, kind="ExternalInput").ap()
    bo = nc.dram_tensor("bo", [E], f32, kind="ExternalInput").ap()
    out = nc.dram_tensor("out", [QC, E], f32, kind="ExternalOutput").ap()

    dbg = {}
    if DEBUG:
        dbg["qT"] = nc.dram_tensor("dbg_qT", [P, NPAIR, QC], bf16, kind="ExternalOutput").ap()
        dbg["kT"] = nc.dram_tensor("dbg_kT", [P, NPAIR, S], bf16, kind="ExternalOutput").ap()
        dbg["v"] = nc.dram_tensor("dbg_v", [P, NKT, H, HD + 1], bf16, kind="ExternalOutput").ap()
        dbg["p"] = nc.dram_tensor("dbg_p", [P, 2, 512], f32, kind="ExternalOutput").ap()
        dbg["sc"] = nc.dram_tensor("dbg_sc", [P, 2, 512], f32, kind="ExternalOutput").ap()
        dbg["attn"] = nc.dram_tensor("dbg_attn", [HD, H, QC], bf16, kind="ExternalOutput").ap()
        dbg["den"] = nc.dram_tensor("dbg_den", [1, 512], f32, kind="ExternalOutput").ap()
        dbg["rep"] = nc.dram_tensor("dbg_rep", [HD, 512], f32, kind="ExternalOutput").ap()

    Exp = mybir.ActivationFunctionType.Exp

    with tile.TileContext(nc) as tc:
        with tc.tile_pool(name="persist", bufs=1) as persist:
            # persistent SBUF tensors (per-partition bytes in comments)
            maskb = persist.tile([P, NKT, QC], bf16)         # 64 KB
            v_all = persist.tile([P, NKT, H, HD + 1], bf16)  # 33.3 KB
            kT_all = persist.tile([P, NPAIR, S], bf16)       # 32 KB
            qT_all = persist.tile([P, NPAIR, QC], bf16)      # 8 KB
            attn_all = persist.tile([HD, H, QC], bf16)       # 16 KB
            wo_sb = persist.tile([HD, H, E], bf16)           # 8 KB
            bo_sb = persist.tile([1, E], f32)

            nc.sync.dma_start(
                out=wo_sb, in_=woT.rearrange("(h d) o -> d h o", d=HD)
            )
            nc.sync.dma_start(out=bo_sb, in_=bo[None, :])

            # ---- mask: bf16 0/1, [k-part, kt, q] ----
            # issue on gpsimd's DMA queue to run parallel with other loads
            for kt in range(NKT):
                nc.gpsimd.dma_start(
                    out=maskb[:, kt, :],
                    in_=maskT[kt * P : (kt + 1) * P, :],
                )

            # ---- v projection: all heads at once ----
            with (
                tc.tile_pool(name="wv", bufs=1) as wvp,
                tc.tile_pool(name="vstage", bufs=3) as vstage,
                tc.tile_pool(name="vps", bufs=2, space="PSUM") as vps,
            ):
                wv_sb = wvp.tile([P, 4, E], bf16)
                nc.sync.dma_start(
                    out=wv_sb, in_=wvT.rearrange("(c p) o -> p c o", p=P)
                )
                for kt in range(NKT):
                    vs = vstage.tile([P, 4, P], bf16)
                    nc.scalar.dma_start(
                        out=vs,
                        in_=valsT[:, kt * P : (kt + 1) * P].rearrange(
                            "(c p) s -> p c s", p=P
                        ),
                    )
                    ps = vps.tile([P, E], f32)
                    for ec in range(4):
                        nc.tensor.matmul(
                            ps,
                            lhsT=vs[:, ec, :],
                            rhs=wv_sb[:, ec, :],
                            start=(ec == 0),
                            stop=(ec == 3),
                        )
                    nc.vector.tensor_copy(
                        out=v_all[:, kt, :, 0:HD],
                        in_=ps.rearrange("p (h d) -> p h d", h=H),
                    )
            # ones column for the softmax denominator
            nc.vector.memset(v_all[:, :, :, HD : HD + 1], 1.0)

            # ---- q projection: [pair-d 128, pair, q] ----
            with (
                tc.tile_pool(name="wq", bufs=1) as wqp,
                tc.tile_pool(name="qstage", bufs=2) as qstage,
                tc.tile_pool(name="qps", bufs=2, space="PSUM") as qps,
            ):
                wq_sb = wqp.tile([P, 4, E], bf16)
                nc.sync.dma_start(
                    out=wq_sb, in_=wqT.rearrange("(c p) o -> p c o", p=P)
                )
                for qt in range(NQT):
                    qs = qstage.tile([P, 4, 512], bf16)
                    nc.scalar.dma_start(
                        out=qs,
                        in_=qryT[:, qt * 512 : (qt + 1) * 512].rearrange(
                            "(c p) s -> p c s", p=P
                        ),
                    )
                    for c in range(NPAIR):
                        ps = qps.tile([P, 512], f32)
                        for ec in range(4):
                            nc.tensor.matmul(
                                ps,
                                lhsT=wq_sb[:, ec, c * P : (c + 1) * P],
                                rhs=qs[:, ec, :],
                                start=(ec == 0),
                                stop=(ec == 3),
                            )
                        nc.vector.tensor_copy(
                            out=qT_all[:, c, qt * 512 : (qt + 1) * 512], in_=ps
                        )

            # ---- k projection (interleaved per pair) + attention ----
            with (
                tc.tile_pool(name="wk", bufs=1) as wkp,
                tc.tile_pool(name="kstage", bufs=2) as kstage,
                tc.tile_pool(name="kps", bufs=2, space="PSUM") as kps,
                tc.tile_pool(name="scps", bufs=2, space="PSUM") as scps,
                tc.tile_pool(name="pvps", bufs=2, space="PSUM") as pvps,
                tc.tile_pool(name="pp", bufs=3) as pp,
                tc.tile_pool(name="norm", bufs=2) as norm,
                tc.tile_pool(name="ndram", bufs=2, space="DRAM") as ndram,
            ):
                wk_sb = wkp.tile([P, 4, E], bf16)
                nc.sync.dma_start(
                    out=wk_sb, in_=wkT.rearrange("(c p) o -> p c o", p=P)
                )
                for c in range(NPAIR):
                    # project kT for this pair (overlaps prev pair's attention)
                    for kt8 in range(S // 512):
                        ks = kstage.tile([P, 4, 512], bf16)
                        nc.sync.dma_start(
                            out=ks,
                            in_=keysT[
                                :, kt8 * 512 : (kt8 + 1) * 512
                            ].rearrange("(c2 p) s -> p c2 s", p=P),
                        )
                        ps = kps.tile([P, 512], f32)
                        for ec in range(4):
                            nc.tensor.matmul(
                                ps,
                                lhsT=wk_sb[:, ec, c * P : (c + 1) * P],
                                rhs=ks[:, ec, :],
                                start=(ec == 0),
                                stop=(ec == 3),
                            )
                        nc.vector.tensor_copy(
                            out=kT_all[:, c, kt8 * 512 : (kt8 + 1) * 512],
                            in_=ps,
                        )
                    for qt in range(NQT):
                        qsl = slice(qt * 512, (qt + 1) * 512)
                        pv0 = pvps.tile([HD + 1, 512], f32, tag="pv")
                        pv1 = pvps.tile([HD + 1, 512], f32, tag="pv")
                        for kt in range(NKT):
                            ksl = slice(kt * P, (kt + 1) * P)
                            sc = scps.tile([P, 2, 512], f32)
                            nc.tensor.matmul(
                                sc[:, 0, :],
                                lhsT=kT_all[0:HD, c, ksl],
                                rhs=qT_all[0:HD, c, qsl],
                                start=True,
                                stop=True,
                            )
                            nc.tensor.matmul(
                                sc[:, 1, :],
                                lhsT=kT_all[HD : 2 * HD, c, ksl],
                                rhs=qT_all[HD : 2 * HD, c, qsl],
                                start=True,
                                stop=True,
                            )
                            p_sb = pp.tile([P, 2, 512], bf16)
                            nc.scalar.activation(p_sb, sc, Exp, scale=SCALE)
                            for s_ in range(2):
                                nc.vector.tensor_tensor(
                                    out=p_sb[:, s_, :],
                                    in0=p_sb[:, s_, :],
                                    in1=maskb[:, kt, qsl],
                                    op=mybir.AluOpType.mult,
                                )
                            if DEBUG and c == 0 and qt == 0 and kt == 0:
                                dt_ = norm.tile([P, 2, 512], f32, tag="den")
                                nc.vector.tensor_copy(out=dt_, in_=p_sb)
                                nc.sync.dma_start(out=dbg["p"], in_=dt_)
                                dt2 = norm.tile([P, 2, 512], f32, tag="den")
                                nc.vector.tensor_copy(out=dt2, in_=sc)
                                nc.sync.dma_start(out=dbg["sc"], in_=dt2)
                            nc.tensor.matmul(
                                pv0,
                                lhsT=v_all[:, kt, 2 * c, :],
                                rhs=p_sb[:, 0, :],
                                start=(kt == 0),
                                stop=(kt == NKT - 1),
                            )
                            nc.tensor.matmul(
                                pv1,
                                lhsT=v_all[:, kt, 2 * c + 1, :],
                                rhs=p_sb[:, 1, :],
                                start=(kt == 0),
                                stop=(kt == NKT - 1),
                            )
                        for s_, pv in ((0, pv0), (1, pv1)):
                            h = 2 * c + s_
                            # copy PV out of PSUM right away (frees the bank
                            # for the next (c, qt) iteration's accumulation)
                            pv_sb = norm.tile([P, 512], f32, tag="den")
                            nc.vector.tensor_copy(
                                out=pv_sb[0 : HD + 1, :],
                                in_=pv[0 : HD + 1, :],
                            )
                            # replicate den across partitions 0..63 via a
                            # DRAM bounce (DRAM sources allow stride-0
                            # partition broadcast APs; SBUF sources don't),
                            # then reciprocal at base partition 0 (the
                            # custom DVE op misbehaves at base 64)
                            dscr = ndram.tile([1, 512], f32, tag="dscr")
                            nc.sync.dma_start(
                                out=dscr, in_=pv_sb[HD : HD + 1, :]
                            )
                            den_rep = norm.tile([HD, 512], f32, tag="denr")
                            nc.sync.dma_start(
                                out=den_rep,
                                in_=bass.AP(
                                    tensor=dscr.tensor,
                                    offset=dscr.offset,
                                    ap=[[0, HD], [1, 512]],
                                ),
                            )
                            rep_sb = norm.tile([HD, 512], f32, tag="rep")
                            nc.vector.reciprocal_approx_fast(
                                out=rep_sb, in_=den_rep
                            )
                            if DEBUG and c == 0 and qt == 0 and s_ == 0:
                                nc.sync.dma_start(out=dbg["den"], in_=pv_sb[HD : HD + 1, :])
                                nc.sync.dma_start(out=dbg["rep"], in_=rep_sb)
                            nc.vector.tensor_tensor(
                                out=attn_all[:, h, qsl],
                                in0=pv_sb[0:HD, :],
                                in1=rep_sb,
                                op=mybir.AluOpType.mult,
                            )

            if DEBUG:
                for name, tile_src in (("qT", qT_all), ("kT", kT_all), ("v", v_all), ("attn", attn_all)):
                    nc.sync.dma_start(out=dbg[name], in_=tile_src)

            # ---- output projection + bias ----
            with (
                tc.tile_pool(name="ops", bufs=2, space="PSUM") as ops,
                tc.tile_pool(name="osb", bufs=3) as osb,
                tc.tile_pool(name="onesp", bufs=1) as onesp,
            ):
                ones1 = onesp.tile([1, P], f32)
                nc.vector.memset(ones1, 1.0)
                for q8 in range(QC // P):
                    ps = ops.tile([P, E], f32)
                    for h in range(H):
                        nc.tensor.matmul(
                            ps,
                            lhsT=attn_all[:, h, q8 * P : (q8 + 1) * P],
                            rhs=wo_sb[:, h, :],
                            start=(h == 0),
                            stop=False,
                        )
                    # bias via rank-1 matmul: ones^T (1x128) @ bo (1x512)
                    nc.tensor.matmul(
                        ps,
                        lhsT=ones1,
                        rhs=bo_sb,
                        start=False,
                        stop=True,
                    )
                    ob = osb.tile([P, E], f32)
                    nc.vector.tensor_copy(out=ob, in_=ps)
                    nc.gpsimd.dma_start(
                        out=out[q8 * P : (q8 + 1) * P, :], in_=ob
                    )

    nc.compile()
    _CACHE["nc"] = nc
    return nc


def make_in_maps(values, keys, query, mask, Wv, Wk, Wq, Wo, bo):
    values = np.asarray(values, np.float32)
    keys = np.asarray(keys, np.float32)
    query = np.asarray(query, np.float32)
    mask = np.asarray(mask)
    wqT = np.ascontiguousarray(np.asarray(Wq, np.float32).T.astype(BF16))
    wkT = np.ascontiguousarray(np.asarray(Wk, np.float32).T.astype(BF16))
    wvT = np.ascontiguousarray(np.asarray(Wv, np.float32).T.astype(BF16))
    woT = np.ascontiguousarray(np.asarray(Wo, np.float32).T.astype(BF16))
    bo = np.ascontiguousarray(np.asarray(bo, np.float32))

    in_maps = []
    for core in range(NCORES):
        b, qc = core // (NCORES // B), core % (NCORES // B)
        qsl = slice(qc * QC, (qc + 1) * QC)
        in_maps.append(
            {
                "maskT": np.ascontiguousarray(
                    mask[b, 0, qsl, :].T.astype(BF16)
                ),
                "keysT": np.ascontiguousarray(keys[b].T.astype(BF16)),
                "valsT": np.ascontiguousarray(values[b].T.astype(BF16)),
                "qryT": np.ascontiguousarray(query[b, qsl].T.astype(BF16)),
                "wqT": wqT,
                "wkT": wkT,
                "wvT": wvT,
                "woT": woT,
                "bo": bo,
            }
        )
    return in_maps


def kernel(values, keys, query, mask, Wv, Wk, Wq, Wo, bo):
    global LAST_RESULT
    from concourse.bass_utils import run_bass_kernel_spmd

    nc = _build()
    in_maps = make_in_maps(values, keys, query, mask, Wv, Wk, Wq, Wo, bo)
    res = run_bass_kernel_spmd(nc, in_maps, core_ids=list(range(NCORES)))
    LAST_RESULT = res

    out = np.empty((B, S, E), np.float32)
    for core in range(NCORES):
        b, qc = core // (NCORES // B), core % (NCORES // B)
        out[b, qc * QC : (qc + 1) * QC] = res.results[core]["out"]
    return out
